# revision 7
# baseline (speedup 1.0000x reference)
"""Trainium2 Bass kernel for a 3-layer edge-featured GAT over 256 dense 84-node graphs.

Contract: kernel(**inputs) takes the FULL unsharded inputs (as produced by the
problem's setup_inputs) and returns the FULL [256, 1] float32 output.

Strategy (data parallel over graphs, 32 graphs/core on 8 cores):
  Each graph is dense (all ordered pairs + self loops), so message passing
  collapses to dense per-graph [84, 84] attention matrices. Host-side we
  scatter edge_attr into dense per-graph planes (folding the per-layer edge
  MLP down to a scalar per edge, and the PyG mean self-loop attr onto the
  diagonal), fold a_src/a_dst into the layer weight ("combined" matmuls), and
  fold the readout linear layer into the last GAT layer (v-columns).

  On device, per layer: one combined matmul gives projected features h~ plus
  per-node attention scalars; the [src, dst] logit plane is accumulated in
  PSUM from the host E plane (identity matmul), a rank-1 broadcast of a_dst,
  and a block-diagonal mask matmul broadcast of a_src; Lrelu+Exp on ScalarE
  give unnormalized attention; per-graph matmuls against node-major h~ (plus a
  ones column) give the aggregate and softmax denominator in one pass, with
  the normalization applied as a batched reciprocal + per-graph scale.
"""

import sys

for _p in ("/opt/trn_rl_repo",):
    if _p not in sys.path:
        sys.path.append(_p)

import numpy as np

from contextlib import ExitStack

from concourse import bacc, bass, mybir, tile
from concourse.bass_utils import run_bass_kernel_spmd

F32 = mybir.dt.float32
AF = mybir.ActivationFunctionType

NPG = 84            # nodes per graph
B = 256             # graphs
HID = 64
DEPTH = 3
NEG_SLOPE = 0.2
NC_CORES = 8
GPC = B // NC_CORES     # 32 graphs per core
NB = GPC * NPG          # 2688 nodes per core
CH = 448                # free-dim chunk (one PSUM bank)
NCH = NB // CH          # 6 chunks


def _host_preprocess(inputs):
    x = np.ascontiguousarray(np.asarray(inputs['x'], np.float32))
    ei = np.asarray(inputs['edge_index'])
    ea = np.asarray(inputs['edge_attr'], np.float32)
    W0 = np.asarray(inputs['W0'], np.float32)
    Ws = np.asarray(inputs['Ws'], np.float32)
    asl = np.asarray(inputs['att_src_all'], np.float32)
    adl = np.asarray(inputs['att_dst_all'], np.float32)
    Wel = np.asarray(inputs['W_edge_all'], np.float32)
    ael = np.asarray(inputs['att_edge_all'], np.float32)
    bl = np.asarray(inputs['bias_all'], np.float32)
    linW = np.asarray(inputs['lin_W'], np.float32)
    linb = np.asarray(inputs['lin_b'], np.float32)

    src, dst = np.asarray(ei[0]), np.asarray(ei[1])
    g = src // NPG
    assert np.all(dst // NPG == g), "edges cross graph boundaries"
    sl, dl = src % NPG, dst % NPG

    dense = np.zeros((B, NPG, NPG, 2), np.float32)
    dense[g, sl, dl] = ea
    cnt = np.zeros((B, NPG), np.float32)
    np.add.at(cnt, (g, dl), 1.0)
    colsum = dense.sum(axis=1)
    loop_attr = colsum / np.maximum(cnt, 1.0)[..., None]
    di = np.arange(NPG)
    dense[:, di, di, :] = loop_attr

    Es = []
    for l in range(DEPTH):
        w2 = Wel[l] @ ael[l]
        Es.append(np.ascontiguousarray(dense @ w2, dtype=np.float32))

    W_all = [W0, Ws[0], Ws[1]]
    CW = []
    for l in range(DEPTH):
        # column order: [a_dst (row 0, base-partition-0 for the rank-1 matmul)
        #                | W (rows 1..64) | a_src (row 65, DMA-bounced) | v]
        cols = [(W_all[l] @ adl[l])[:, None], W_all[l], (W_all[l] @ asl[l])[:, None]]
        if l == DEPTH - 1:
            cols.append(W_all[l] @ linW)
        CW.append(np.ascontiguousarray(np.concatenate(cols, axis=1), np.float32))

    tail_bias = float(NPG * float(bl[DEPTH - 1] @ linW[:, 0]) + float(linb[0]))

    mask = np.zeros((GPC, NB), np.float32)
    for gg in range(GPC):
        mask[gg, gg * NPG:(gg + 1) * NPG] = 1.0
    ident = np.eye(NPG, dtype=np.float32)
    ones = np.ones((NPG, NPG), np.float32)

    return dict(x=x, Es=Es, CW=CW, bl=bl, tail_bias=tail_bias,
                mask=mask, ident=ident, ones=ones)


def _graph_banks(n_graphs, per_bank):
    out = []
    g0 = 0
    while g0 < n_graphs:
        out.append(list(range(g0, min(g0 + per_bank, n_graphs))))
        g0 += per_bank
    return out


def _build_program(tail_bias, use_bias):
    """use_bias: (bool, bool) for layers 0 and 1 (per-node bias via ex@bb matmul)."""
    nc = bacc.Bacc("TRN2", target_bir_lowering=False, debug=False)

    xT_d = nc.dram_tensor("xT", [1, NB], F32, kind="ExternalInput").ap()
    E_d = [nc.dram_tensor(f"E{l}", [NPG, NB], F32, kind="ExternalInput").ap()
           for l in range(DEPTH)]
    CW_d = [nc.dram_tensor(f"CW{l}", [(1 if l == 0 else HID), 66 + (1 if l == 2 else 0)],
                           F32, kind="ExternalInput").ap() for l in range(DEPTH)]
    mask_d = nc.dram_tensor("mask", [GPC, NB], F32, kind="ExternalInput").ap()
    ident_d = nc.dram_tensor("ident", [NPG, NPG], F32, kind="ExternalInput").ap()
    ones_d = nc.dram_tensor("ones", [NPG, NPG], F32, kind="ExternalInput").ap()
    bb_d = [nc.dram_tensor(f"bb{l}", [NPG, HID], F32, kind="ExternalInput").ap()
            if use_bias[l] else None for l in range(2)]
    # row bounce scratch (sbuf row -> dram -> repartitioned sbuf)
    asrc_tmp = [nc.dram_tensor(f"asrc_tmp{l}", [NB], F32).ap() for l in range(DEPTH)]
    v_tmp = nc.dram_tensor("v_tmp", [NB], F32).ap()
    out_d = nc.dram_tensor("out", [GPC], F32, kind="ExternalOutput").ap()

    with tile.TileContext(nc) as tc, ExitStack() as ctx:
        cpool = ctx.enter_context(tc.tile_pool(name="const", bufs=1))
        hpool = ctx.enter_context(tc.tile_pool(name="h", bufs=2))
        ppool = ctx.enter_context(tc.tile_pool(name="proj", bufs=2))
        npool = ctx.enter_context(tc.tile_pool(name="hnode", bufs=2))
        expool = ctx.enter_context(tc.tile_pool(name="ex", bufs=2))
        echpool = ctx.enter_context(tc.tile_pool(name="ech", bufs=3))
        ltpool = ctx.enter_context(tc.tile_pool(name="lt", bufs=2))
        smpool = ctx.enter_context(tc.tile_pool(name="small", bufs=2))
        upool = ctx.enter_context(tc.tile_pool(name="u", bufs=2))

        psw = ctx.enter_context(tc.tile_pool(name="psw", bufs=2, space="PSUM"))
        psl = ctx.enter_context(tc.tile_pool(name="psl", bufs=2, space="PSUM"))
        psa = ctx.enter_context(tc.tile_pool(name="psa", bufs=2, space="PSUM"))
        pst = ctx.enter_context(tc.tile_pool(name="pst", bufs=1, space="PSUM"))
        pst2 = ctx.enter_context(tc.tile_pool(name="pst2", bufs=1, space="PSUM"))

        # constants
        mask_sb = cpool.tile([GPC, NB], F32, tag="mask")
        nc.sync.dma_start(mask_sb[:], mask_d[:])
        ident_sb = cpool.tile([NPG, NPG], F32, tag="ident")
        nc.sync.dma_start(ident_sb[:], ident_d[:])
        ones_sb = cpool.tile([NPG, NPG], F32, tag="ones")
        nc.sync.dma_start(ones_sb[:], ones_d[:])
        cw_sb = []
        for l in range(DEPTH):
            t = cpool.tile(list(CW_d[l].shape), F32, tag=f"cw{l}")
            nc.sync.dma_start(t[:], CW_d[l][:])
            cw_sb.append(t)
        bb_sb = []
        for l in range(2):
            if use_bias[l]:
                t = cpool.tile([NPG, HID], F32, tag=f"bb{l}")
                nc.sync.dma_start(t[:], bb_d[l][:])
                bb_sb.append(t)
            else:
                bb_sb.append(None)

        x_sb = hpool.tile([1, NB], F32, tag="x")
        nc.sync.dma_start(x_sb[:], xT_d[:])

        hT_in = x_sb
        for l in range(2):
            ncols = 66
            # 1) combined projection: [ncols, NB] = CW^T @ hT_in
            pT66 = ppool.tile([ncols, NB], F32, tag="pT66")
            for c in range(NCH):
                cs = slice(c * CH, (c + 1) * CH)
                pw = psw.tile([ncols, CH], F32, tag="pw")
                nc.tensor.matmul(pw[:], cw_sb[l][:], hT_in[:, cs],
                                 start=True, stop=True)
                nc.scalar.copy(pT66[:, cs], pw[:])
            # 2) bounce asrc row -> [GPC, NPG]
            nc.sync.dma_start(asrc_tmp[l].rearrange("(o n) -> o n", o=1),
                              pT66[65:66, :])
            asrc_mat = smpool.tile([GPC, NPG], F32, tag="asrc")
            nc.sync.dma_start(asrc_mat[:],
                              asrc_tmp[l].rearrange("(g s) -> g s", g=GPC))
            # 3) per-graph transposes of h~ -> node-major [NPG, GPC*66]
            hnode = npool.tile([NPG, GPC * 66], F32, tag="hnode")
            for gs in _graph_banks(GPC, 7):
                pt = pst.tile([NPG, 66 * len(gs)], F32, tag="pt")
                for j, g in enumerate(gs):
                    nc.tensor.transpose(pt[:, j * 66:(j + 1) * 66],
                                        pT66[:, g * NPG:(g + 1) * NPG],
                                        ident_sb[:66, :66])
                nc.scalar.copy(hnode[:, gs[0] * 66:(gs[-1] + 1) * 66], pt[:])
            # 4) logits psum + lrelu + exp per chunk
            ex = expool.tile([NPG, NB], F32, tag="ex")
            for c in range(NCH):
                cs = slice(c * CH, (c + 1) * CH)
                ech = echpool.tile([NPG, CH], F32, tag="ech")
                nc.sync.dma_start(ech[:], E_d[l][:, cs])
                pl = psl.tile([NPG, CH], F32, tag="pl")
                nc.tensor.matmul(pl[:], ident_sb[:], ech[:],
                                 start=True, stop=False)
                nc.tensor.matmul(pl[:], ones_sb[0:1, :], pT66[0:1, cs],
                                 start=False, stop=False)
                nc.tensor.matmul(pl[:], asrc_mat[:], mask_sb[:, cs],
                                 start=False, stop=True)
                lt0 = ltpool.tile([NPG, CH], F32, tag="lt0")
                nc.scalar.copy(lt0[:], pl[:])
                lt = ltpool.tile([NPG, CH], F32, tag="lt")
                nc.vector.scalar_tensor_tensor(lt[:], lt0[:], NEG_SLOPE, lt0[:],
                                               mybir.AluOpType.mult,
                                               mybir.AluOpType.max)
                nc.scalar.activation(ex[:, cs], lt[:], AF.Exp)
            # 5) per-graph aggregation (+ softmax denominator column)
            U = upool.tile([NPG, GPC * 65], F32, tag="U")
            for gs in _graph_banks(GPC, 7):
                pa = psa.tile([NPG, 65 * len(gs)], F32, tag="pa")
                for j, g in enumerate(gs):
                    exg = ex[:, g * NPG:(g + 1) * NPG]
                    o0 = j * 65
                    nc.tensor.matmul(pa[:, o0:o0 + 64], exg,
                                     hnode[:, g * 66 + 1:g * 66 + 65],
                                     start=True, stop=not use_bias[l])
                    if use_bias[l]:
                        nc.tensor.matmul(pa[:, o0:o0 + 64], exg, bb_sb[l][:],
                                         start=False, stop=True)
                    nc.tensor.matmul(pa[:, o0 + 64:o0 + 65], exg,
                                     ones_sb[:, 0:1], start=True, stop=True)
                nc.scalar.activation(U[:, gs[0] * 65:(gs[-1] + 1) * 65], pa[:],
                                     AF.Relu)
            # 6) normalize: U / den, per graph
            dent = smpool.tile([NPG, GPC], F32, tag="dent")
            nc.vector.tensor_copy(dent[:], U[:, 64::65])
            recip = smpool.tile([NPG, GPC], F32, tag="recip")
            nc.vector.reciprocal(recip[:], dent[:])
            UN = upool.tile([NPG, GPC * HID], F32, tag="UN")
            for g in range(GPC):
                nc.vector.tensor_scalar_mul(UN[:, g * HID:(g + 1) * HID],
                                            U[:, g * 65:g * 65 + 64],
                                            recip[:, g:g + 1])
            # 7) transpose back to feature-major hT for the next layer
            hT_next = hpool.tile([HID, NB], F32, tag="hT")
            for gs in _graph_banks(GPC, 6):
                pt2 = pst2.tile([HID, NPG * len(gs)], F32, tag="pt2")
                for j, g in enumerate(gs):
                    nc.tensor.transpose(pt2[:, j * NPG:(j + 1) * NPG],
                                        UN[:, g * HID:(g + 1) * HID],
                                        ident_sb[:])
                nc.scalar.copy(hT_next[:, gs[0] * NPG:(gs[-1] + 1) * NPG],
                               pt2[:])
            hT_in = hT_next

        # ---- layer 2 (readout folded in) ----
        pT67 = ppool.tile([67, NB], F32, tag="pT66")
        for c in range(NCH):
            cs = slice(c * CH, (c + 1) * CH)
            pw = psw.tile([67, CH], F32, tag="pw")
            nc.tensor.matmul(pw[:], cw_sb[2][:], hT_in[:, cs],
                             start=True, stop=True)
            nc.scalar.copy(pT67[:, cs], pw[:])
        nc.sync.dma_start(asrc_tmp[2].rearrange("(o n) -> o n", o=1),
                          pT67[65:66, :])
        asrc_mat = smpool.tile([GPC, NPG], F32, tag="asrc")
        nc.sync.dma_start(asrc_mat[:],
                          asrc_tmp[2].rearrange("(g s) -> g s", g=GPC))
        nc.sync.dma_start(v_tmp.rearrange("(o n) -> o n", o=1), pT67[66:67, :])
        v_mat = smpool.tile([NPG, GPC], F32, tag="vmat")
        nc.sync.dma_start(v_mat[:], v_tmp.rearrange("(g s) -> s g", g=GPC))
        # interleaved [v | 1] columns so each graph aggregates with one matmul
        vo = smpool.tile([NPG, 2 * GPC], F32, tag="vo")
        nc.vector.memset(vo[:], 1.0)
        nc.vector.tensor_copy(vo[:, 0::2], v_mat[:])

        ex = expool.tile([NPG, NB], F32, tag="ex")
        for c in range(NCH):
            cs = slice(c * CH, (c + 1) * CH)
            ech = echpool.tile([NPG, CH], F32, tag="ech")
            nc.sync.dma_start(ech[:], E_d[2][:, cs])
            pl = psl.tile([NPG, CH], F32, tag="pl")
            nc.tensor.matmul(pl[:], ident_sb[:], ech[:], start=True, stop=False)
            nc.tensor.matmul(pl[:], ones_sb[0:1, :], pT67[0:1, cs],
                             start=False, stop=False)
            nc.tensor.matmul(pl[:], asrc_mat[:], mask_sb[:, cs],
                             start=False, stop=True)
            lt0 = ltpool.tile([NPG, CH], F32, tag="lt0")
            nc.scalar.copy(lt0[:], pl[:])
            lt = ltpool.tile([NPG, CH], F32, tag="lt")
            nc.vector.scalar_tensor_tensor(lt[:], lt0[:], NEG_SLOPE, lt0[:],
                                           mybir.AluOpType.mult,
                                           mybir.AluOpType.max)
            nc.scalar.activation(ex[:, cs], lt[:], AF.Exp)

        pq = psa.tile([NPG, 2 * GPC], F32, tag="pa")
        for g in range(GPC):
            nc.tensor.matmul(pq[:, 2 * g:2 * g + 2],
                             ex[:, g * NPG:(g + 1) * NPG],
                             vo[:, 2 * g:2 * g + 2], start=True, stop=True)
        Q = smpool.tile([NPG, 2 * GPC], F32, tag="Q")
        nc.scalar.copy(Q[:], pq[:])
        den2 = smpool.tile([NPG, GPC], F32, tag="dent")
        nc.vector.tensor_copy(den2[:], Q[:, 1::2])
        recip2 = smpool.tile([NPG, GPC], F32, tag="recip")
        nc.vector.reciprocal(recip2[:], den2[:])
        qsb = smpool.tile([NPG, GPC], F32, tag="qsb")
        nc.vector.tensor_mul(qsb[:], Q[:, 0::2], recip2[:])
        pz = pst.tile([1, GPC], F32, tag="pt")
        nc.tensor.matmul(pz[:], ones_sb[:, 0:1], qsb[:], start=True, stop=True)
        z_sb = smpool.tile([1, GPC], F32, tag="z")
        nc.scalar.activation(z_sb[:], pz[:], AF.Relu, bias=float(tail_bias))
        nc.sync.dma_start(out_d.rearrange("(o g) -> o g", o=1), z_sb[:])

    nc.compile()
    return nc


def _core_inputs(pre, c):
    m = {
        'xT': np.ascontiguousarray(
            pre['x'][c * NB:(c + 1) * NB, 0].reshape(1, NB)),
        'mask': pre['mask'], 'ident': pre['ident'], 'ones': pre['ones'],
    }
    for l in range(DEPTH):
        m[f'E{l}'] = np.ascontiguousarray(
            np.transpose(pre['Es'][l][c * GPC:(c + 1) * GPC], (1, 0, 2))
            .reshape(NPG, NB))
        m[f'CW{l}'] = pre['CW'][l]
    for l in range(2):
        if np.any(pre['bl'][l] != 0):
            m[f'bb{l}'] = np.ascontiguousarray(
                np.tile(pre['bl'][l][None, :], (NPG, 1)))
    return m


def kernel(**inputs):
    pre = _host_preprocess(inputs)
    use_bias = tuple(bool(np.any(pre['bl'][l] != 0)) for l in range(2))
    nc = _build_program(pre['tail_bias'], use_bias)
    in_maps = [_core_inputs(pre, c) for c in range(NC_CORES)]
    res = run_bass_kernel_spmd(nc, in_maps, list(range(NC_CORES)))
    out = np.concatenate([np.asarray(res.results[c]['out'])
                          for c in range(NC_CORES)])
    return out.reshape(B, 1).astype(np.float32)


if __name__ == "__main__":
    rng = np.random.default_rng(0)
    print("kernel.py loaded")


# revision 10
# speedup vs baseline: 1.9533x; 1.9533x over previous
"""Trainium2 Bass kernel for a 3-layer edge-featured GAT over 256 dense 84-node graphs.

Contract: kernel(**inputs) takes the FULL unsharded inputs (as produced by the
problem's setup_inputs) and returns the FULL [256, 1] float32 output.

Strategy (data parallel over graphs, 32 graphs/core on 8 cores):
  Each graph is dense (all ordered pairs + self loops), so message passing
  collapses to dense per-graph [84, 84] attention matrices. Host-side we
  scatter edge_attr into dense per-graph planes (folding the per-layer edge
  MLP down to a scalar per edge, and the PyG mean self-loop attr onto the
  diagonal), fold a_src/a_dst into the layer weight ("combined" matmuls), and
  fold the readout linear layer into the last GAT layer (v-column).

  On device, per layer: one combined matmul gives projected features h~ plus
  per-node attention scalars; the [src, dst] logit plane is accumulated in
  PSUM from the host E plane (identity matmul), a rank-1 broadcast of a_dst,
  and a block-diagonal mask matmul broadcast of a_src; lrelu (DVE max) + Exp
  on ScalarE give unnormalized attention; per-graph matmuls against
  node-major h~ (plus a ones column) give the aggregate and softmax
  denominator in one pass, with the normalization applied as a batched
  reciprocal + per-graph scale.

  All matmul operands are fp16 (PSUM accumulation stays fp32): fp32 matmuls
  on TRN2 run 2-pass LOW_HIGH at 4 cycles/row, fp16 runs 1 cycle/row.
  Measured end-to-end error of the fp16 config vs the fp32 reference:
  ~3.5e-4 scale-relative.
"""

import sys

for _p in ("/opt/trn_rl_repo",):
    if _p not in sys.path:
        sys.path.append(_p)

import numpy as np

from contextlib import ExitStack

from concourse import bacc, bass, mybir, tile
from concourse.bass_utils import run_bass_kernel_spmd

F32 = mybir.dt.float32
F16 = mybir.dt.float16
AF = mybir.ActivationFunctionType
ALU = mybir.AluOpType

NPG = 84            # nodes per graph
B = 256             # graphs
HID = 64
DEPTH = 3
NEG_SLOPE = 0.2
NC_CORES = 8
GPC = B // NC_CORES     # 32 graphs per core
NB = GPC * NPG          # 2688 nodes per core
CH = 448                # free-dim chunk (one PSUM bank)
NCH = NB // CH          # 6 chunks


def _host_preprocess(inputs):
    x = np.ascontiguousarray(np.asarray(inputs['x'], np.float32))
    ei = np.asarray(inputs['edge_index'])
    ea = np.asarray(inputs['edge_attr'], np.float32)
    W0 = np.asarray(inputs['W0'], np.float32)
    Ws = np.asarray(inputs['Ws'], np.float32)
    asl = np.asarray(inputs['att_src_all'], np.float32)
    adl = np.asarray(inputs['att_dst_all'], np.float32)
    Wel = np.asarray(inputs['W_edge_all'], np.float32)
    ael = np.asarray(inputs['att_edge_all'], np.float32)
    bl = np.asarray(inputs['bias_all'], np.float32)
    linW = np.asarray(inputs['lin_W'], np.float32)
    linb = np.asarray(inputs['lin_b'], np.float32)

    src, dst = np.asarray(ei[0]), np.asarray(ei[1])
    g = src // NPG
    assert np.all(dst // NPG == g), "edges cross graph boundaries"
    sl, dl = src % NPG, dst % NPG

    dense = np.zeros((B, NPG, NPG, 2), np.float32)
    dense[g, sl, dl] = ea
    cnt = np.zeros((B, NPG), np.float32)
    np.add.at(cnt, (g, dl), 1.0)
    colsum = dense.sum(axis=1)
    loop_attr = colsum / np.maximum(cnt, 1.0)[..., None]
    di = np.arange(NPG)
    dense[:, di, di, :] = loop_attr

    Es = []
    for l in range(DEPTH):
        w2 = Wel[l] @ ael[l]
        Es.append(np.ascontiguousarray(dense @ w2, dtype=np.float16))

    W_all = [W0, Ws[0], Ws[1]]
    CW = []
    for l in range(DEPTH):
        # column order: [a_dst (row 0, base-partition-0 for the rank-1 matmul)
        #                | W (rows 1..64) | a_src (row 65, DMA-bounced) | v]
        cols = [(W_all[l] @ adl[l])[:, None], W_all[l], (W_all[l] @ asl[l])[:, None]]
        if l == DEPTH - 1:
            cols.append(W_all[l] @ linW)
        CW.append(np.ascontiguousarray(np.concatenate(cols, axis=1), np.float16))

    tail_bias = float(NPG * float(bl[DEPTH - 1] @ linW[:, 0]) + float(linb[0]))

    mask = np.zeros((GPC, NB), np.float16)
    for gg in range(GPC):
        mask[gg, gg * NPG:(gg + 1) * NPG] = 1.0
    ident = np.eye(NPG, dtype=np.float16)
    ones = np.ones((NPG, NPG), np.float16)

    return dict(x=x.astype(np.float16), Es=Es, CW=CW, bl=bl,
                tail_bias=tail_bias, mask=mask, ident=ident, ones=ones)


def _graph_banks(n_graphs, per_bank):
    out = []
    g0 = 0
    while g0 < n_graphs:
        out.append(list(range(g0, min(g0 + per_bank, n_graphs))))
        g0 += per_bank
    return out


def _build_program(tail_bias, use_bias):
    """use_bias: (bool, bool) for layers 0 and 1 (per-node bias via ex@bb matmul)."""
    nc = bacc.Bacc("TRN2", target_bir_lowering=False, debug=False)

    xT_d = nc.dram_tensor("xT", [1, NB], F16, kind="ExternalInput").ap()
    E_d = [nc.dram_tensor(f"E{l}", [NPG, NB], F16, kind="ExternalInput").ap()
           for l in range(DEPTH)]
    CW_d = [nc.dram_tensor(f"CW{l}", [(1 if l == 0 else HID), 66 + (1 if l == 2 else 0)],
                           F16, kind="ExternalInput").ap() for l in range(DEPTH)]
    mask_d = nc.dram_tensor("mask", [GPC, NB], F16, kind="ExternalInput").ap()
    ident_d = nc.dram_tensor("ident", [NPG, NPG], F16, kind="ExternalInput").ap()
    ones_d = nc.dram_tensor("ones", [NPG, NPG], F16, kind="ExternalInput").ap()
    bb_d = [nc.dram_tensor(f"bb{l}", [NPG, HID], F16, kind="ExternalInput").ap()
            if use_bias[l] else None for l in range(2)]
    # row bounce scratch (sbuf row -> dram -> repartitioned sbuf)
    asrc_tmp = [nc.dram_tensor(f"asrc_tmp{l}", [NB], F16).ap() for l in range(DEPTH)]
    v_tmp = nc.dram_tensor("v_tmp", [NB], F16).ap()
    q_tmp = nc.dram_tensor("q_tmp", [NPG * GPC], F32).ap()
    out_d = nc.dram_tensor("out", [GPC], F32, kind="ExternalOutput").ap()

    with tile.TileContext(nc) as tc, ExitStack() as ctx:
        cpool = ctx.enter_context(tc.tile_pool(name="const", bufs=1))
        hpool = ctx.enter_context(tc.tile_pool(name="h", bufs=2))
        ppool = ctx.enter_context(tc.tile_pool(name="proj", bufs=2))
        npool = ctx.enter_context(tc.tile_pool(name="hnode", bufs=2))
        expool = ctx.enter_context(tc.tile_pool(name="ex", bufs=2))
        echpool = ctx.enter_context(tc.tile_pool(name="ech", bufs=3))
        ltpool = ctx.enter_context(tc.tile_pool(name="lt", bufs=2))
        smpool = ctx.enter_context(tc.tile_pool(name="small", bufs=2))
        upool = ctx.enter_context(tc.tile_pool(name="u", bufs=2))

        psw = ctx.enter_context(tc.tile_pool(name="psw", bufs=2, space="PSUM"))
        psl = ctx.enter_context(tc.tile_pool(name="psl", bufs=2, space="PSUM"))
        psa = ctx.enter_context(tc.tile_pool(name="psa", bufs=2, space="PSUM"))
        pst = ctx.enter_context(tc.tile_pool(name="pst", bufs=1, space="PSUM"))
        pst2 = ctx.enter_context(tc.tile_pool(name="pst2", bufs=1, space="PSUM"))

        # constants
        mask_sb = cpool.tile([GPC, NB], F16, tag="mask")
        nc.sync.dma_start(mask_sb[:], mask_d[:])
        ident_sb = cpool.tile([NPG, NPG], F16, tag="ident")
        nc.sync.dma_start(ident_sb[:], ident_d[:])
        ones_sb = cpool.tile([NPG, NPG], F16, tag="ones")
        nc.sync.dma_start(ones_sb[:], ones_d[:])
        cw_sb = []
        for l in range(DEPTH):
            t = cpool.tile(list(CW_d[l].shape), F16, tag=f"cw{l}")
            nc.sync.dma_start(t[:], CW_d[l][:])
            cw_sb.append(t)
        bb_sb = []
        for l in range(2):
            if use_bias[l]:
                t = cpool.tile([NPG, HID], F16, tag=f"bb{l}")
                nc.sync.dma_start(t[:], bb_d[l][:])
                bb_sb.append(t)
            else:
                bb_sb.append(None)

        x_sb = hpool.tile([1, NB], F16, tag="x")
        nc.sync.dma_start(x_sb[:], xT_d[:])

        hT_in = x_sb
        for l in range(2):
            # 1) combined projection: [66, NB] = CW^T @ hT_in
            pT66 = ppool.tile([66, NB], F16, tag="pT66")
            for c in range(NCH):
                cs = slice(c * CH, (c + 1) * CH)
                pw = psw.tile([66, CH], F32, tag="pw")
                nc.tensor.matmul(pw[:], cw_sb[l][:], hT_in[:, cs],
                                 start=True, stop=True)
                nc.scalar.copy(pT66[:, cs], pw[:])
            # 2) bounce asrc row -> [GPC, NPG]
            nc.sync.dma_start(asrc_tmp[l].rearrange("(o n) -> o n", o=1),
                              pT66[65:66, :])
            asrc_mat = smpool.tile([GPC, NPG], F16, tag="asrc")
            nc.sync.dma_start(asrc_mat[:],
                              asrc_tmp[l].rearrange("(g s) -> g s", g=GPC))
            # 3) per-graph transposes of h~ -> node-major [NPG, GPC*66]
            hnode = npool.tile([NPG, GPC * 66], F16, tag="hnode")
            for gs in _graph_banks(GPC, 7):
                pt = pst.tile([NPG, 66 * len(gs)], F16, tag="pt")
                for j, g in enumerate(gs):
                    nc.tensor.transpose(pt[:, j * 66:(j + 1) * 66],
                                        pT66[:, g * NPG:(g + 1) * NPG],
                                        ident_sb[:66, :66])
                nc.scalar.copy(hnode[:, gs[0] * 66:(gs[-1] + 1) * 66], pt[:])
            # 4) logits psum + lrelu + exp per chunk
            ex = expool.tile([NPG, NB], F16, tag="ex")
            for c in range(NCH):
                cs = slice(c * CH, (c + 1) * CH)
                ech = echpool.tile([NPG, CH], F16, tag="ech")
                nc.sync.dma_start(ech[:], E_d[l][:, cs])
                pl = psl.tile([NPG, CH], F32, tag="pl")
                nc.tensor.matmul(pl[:], ident_sb[:], ech[:],
                                 start=True, stop=False)
                nc.tensor.matmul(pl[:], ones_sb[0:1, :], pT66[0:1, cs],
                                 start=False, stop=False)
                nc.tensor.matmul(pl[:], asrc_mat[:], mask_sb[:, cs],
                                 start=False, stop=True)
                lt0 = ltpool.tile([NPG, CH], F32, tag="lt0")
                nc.scalar.copy(lt0[:], pl[:])
                lt = ltpool.tile([NPG, CH], F32, tag="lt")
                nc.vector.scalar_tensor_tensor(lt[:], lt0[:], NEG_SLOPE, lt0[:],
                                               ALU.mult, ALU.max)
                nc.scalar.activation(ex[:, cs], lt[:], AF.Exp)
            # 5) per-graph aggregation (+ softmax denominator column)
            U = upool.tile([NPG, GPC * 65], F32, tag="U")
            for gs in _graph_banks(GPC, 7):
                pa = psa.tile([NPG, 65 * len(gs)], F32, tag="pa")
                for j, g in enumerate(gs):
                    exg = ex[:, g * NPG:(g + 1) * NPG]
                    o0 = j * 65
                    nc.tensor.matmul(pa[:, o0:o0 + 64], exg,
                                     hnode[:, g * 66 + 1:g * 66 + 65],
                                     start=True, stop=not use_bias[l])
                    if use_bias[l]:
                        nc.tensor.matmul(pa[:, o0:o0 + 64], exg, bb_sb[l][:],
                                         start=False, stop=True)
                    nc.tensor.matmul(pa[:, o0 + 64:o0 + 65], exg,
                                     ones_sb[:, 0:1], start=True, stop=True)
                nc.scalar.activation(U[:, gs[0] * 65:(gs[-1] + 1) * 65], pa[:],
                                     AF.Relu)
            # 6) normalize: U / den, per graph
            dent = smpool.tile([NPG, GPC], F32, tag="dent")
            nc.vector.tensor_copy(dent[:], U[:, 64::65])
            recip = smpool.tile([NPG, GPC], F32, tag="recip")
            nc.vector.reciprocal(recip[:], dent[:])
            UN = upool.tile([NPG, GPC * HID], F16, tag="UN")
            for g in range(GPC):
                nc.vector.tensor_scalar_mul(UN[:, g * HID:(g + 1) * HID],
                                            U[:, g * 65:g * 65 + 64],
                                            recip[:, g:g + 1])
            # 7) transpose back to feature-major hT for the next layer
            hT_next = hpool.tile([HID, NB], F16, tag="hT")
            for gs in _graph_banks(GPC, 6):
                pt2 = pst2.tile([HID, NPG * len(gs)], F16, tag="pt2")
                for j, g in enumerate(gs):
                    nc.tensor.transpose(pt2[:, j * NPG:(j + 1) * NPG],
                                        UN[:, g * HID:(g + 1) * HID],
                                        ident_sb[:])
                nc.scalar.copy(hT_next[:, gs[0] * NPG:(gs[-1] + 1) * NPG],
                               pt2[:])
            hT_in = hT_next

        # ---- layer 2 (readout folded in) ----
        pT67 = ppool.tile([67, NB], F16, tag="pT66")
        for c in range(NCH):
            cs = slice(c * CH, (c + 1) * CH)
            pw = psw.tile([67, CH], F32, tag="pw")
            nc.tensor.matmul(pw[:], cw_sb[2][:], hT_in[:, cs],
                             start=True, stop=True)
            nc.scalar.copy(pT67[:, cs], pw[:])
        nc.sync.dma_start(asrc_tmp[2].rearrange("(o n) -> o n", o=1),
                          pT67[65:66, :])
        asrc_mat = smpool.tile([GPC, NPG], F16, tag="asrc")
        nc.sync.dma_start(asrc_mat[:],
                          asrc_tmp[2].rearrange("(g s) -> g s", g=GPC))
        nc.sync.dma_start(v_tmp.rearrange("(o n) -> o n", o=1), pT67[66:67, :])
        v_mat = smpool.tile([NPG, GPC], F16, tag="vmat")
        nc.sync.dma_start(v_mat[:], v_tmp.rearrange("(g s) -> s g", g=GPC))
        # interleaved [v | 1] columns so each graph aggregates with one matmul
        vo = smpool.tile([NPG, 2 * GPC], F16, tag="vo")
        nc.vector.memset(vo[:], 1.0)
        nc.vector.tensor_copy(vo[:, 0::2], v_mat[:])

        ex = expool.tile([NPG, NB], F16, tag="ex")
        for c in range(NCH):
            cs = slice(c * CH, (c + 1) * CH)
            ech = echpool.tile([NPG, CH], F16, tag="ech")
            nc.sync.dma_start(ech[:], E_d[2][:, cs])
            pl = psl.tile([NPG, CH], F32, tag="pl")
            nc.tensor.matmul(pl[:], ident_sb[:], ech[:], start=True, stop=False)
            nc.tensor.matmul(pl[:], ones_sb[0:1, :], pT67[0:1, cs],
                             start=False, stop=False)
            nc.tensor.matmul(pl[:], asrc_mat[:], mask_sb[:, cs],
                             start=False, stop=True)
            lt0 = ltpool.tile([NPG, CH], F32, tag="lt0")
            nc.scalar.copy(lt0[:], pl[:])
            lt = ltpool.tile([NPG, CH], F32, tag="lt")
            nc.vector.scalar_tensor_tensor(lt[:], lt0[:], NEG_SLOPE, lt0[:],
                                           ALU.mult, ALU.max)
            nc.scalar.activation(ex[:, cs], lt[:], AF.Exp)

        pq = psa.tile([NPG, 2 * GPC], F32, tag="pa")
        for g in range(GPC):
            nc.tensor.matmul(pq[:, 2 * g:2 * g + 2],
                             ex[:, g * NPG:(g + 1) * NPG],
                             vo[:, 2 * g:2 * g + 2], start=True, stop=True)
        Q = smpool.tile([NPG, 2 * GPC], F32, tag="Q")
        nc.scalar.copy(Q[:], pq[:])
        den2 = smpool.tile([NPG, GPC], F32, tag="dent")
        nc.vector.tensor_copy(den2[:], Q[:, 1::2])
        recip2 = smpool.tile([NPG, GPC], F32, tag="recip")
        nc.vector.reciprocal(recip2[:], den2[:])
        qsb = smpool.tile([NPG, GPC], F32, tag="qsb")
        nc.vector.tensor_mul(qsb[:], Q[:, 0::2], recip2[:])
        # exact fp32 pooling: bounce [d, g] -> [g, d], then free-axis reduce
        nc.sync.dma_start(q_tmp.rearrange("(s g) -> s g", g=GPC), qsb[:])
        qT = smpool.tile([GPC, NPG], F32, tag="qT")
        nc.sync.dma_start(qT[:], q_tmp.rearrange("(s g) -> g s", g=GPC))
        zcol = smpool.tile([GPC, 1], F32, tag="zcol")
        nc.vector.reduce_sum(zcol[:], qT[:], axis=mybir.AxisListType.X)
        zout = smpool.tile([GPC, 1], F32, tag="zout")
        nc.scalar.activation(zout[:], zcol[:], AF.Relu, bias=float(tail_bias))
        nc.sync.dma_start(out_d.rearrange("(g o) -> g o", o=1), zout[:])

    nc.compile()
    return nc


def _core_inputs(pre, c):
    m = {
        'xT': np.ascontiguousarray(
            pre['x'][c * NB:(c + 1) * NB, 0].reshape(1, NB)),
        'mask': pre['mask'], 'ident': pre['ident'], 'ones': pre['ones'],
    }
    for l in range(DEPTH):
        m[f'E{l}'] = np.ascontiguousarray(
            np.transpose(pre['Es'][l][c * GPC:(c + 1) * GPC], (1, 0, 2))
            .reshape(NPG, NB))
        m[f'CW{l}'] = pre['CW'][l]
    for l in range(2):
        if np.any(pre['bl'][l] != 0):
            m[f'bb{l}'] = np.ascontiguousarray(
                np.tile(pre['bl'][l][None, :], (NPG, 1)).astype(np.float16))
    return m


def kernel(**inputs):
    pre = _host_preprocess(inputs)
    use_bias = tuple(bool(np.any(pre['bl'][l] != 0)) for l in range(2))
    nc = _build_program(pre['tail_bias'], use_bias)
    in_maps = [_core_inputs(pre, c) for c in range(NC_CORES)]
    res = run_bass_kernel_spmd(nc, in_maps, list(range(NC_CORES)))
    out = np.concatenate([np.asarray(res.results[c]['out'])
                          for c in range(NC_CORES)])
    return out.reshape(B, 1).astype(np.float32)


# revision 12
# speedup vs baseline: 2.2278x; 1.1405x over previous
"""Trainium2 Bass kernel for a 3-layer edge-featured GAT over 256 dense 84-node graphs.

Contract: kernel(**inputs) takes the FULL unsharded inputs (as produced by the
problem's setup_inputs) and returns the FULL [256, 1] float32 output.

Strategy (data parallel over graphs, 32 graphs/core on 8 cores):
  Each graph is dense (all ordered pairs + self loops), so message passing
  collapses to dense per-graph [84, 84] attention matrices. Host-side we
  scatter edge_attr into dense per-graph planes (folding the per-layer edge
  MLP down to a scalar per edge, and the PyG mean self-loop attr onto the
  diagonal), fold a_src/a_dst/readout into augmented layer weights, and keep
  a constant-one input feature so every projection carries a ones column
  (which turns the softmax denominator into one extra matmul column).

  Per layer on device: one combined projection produces, per node, the
  projected features h~, a_src/a_dst attention scalars and a constant 1;
  the [src, dst] logit plane is accumulated in PSUM from the host E plane
  (identity matmul), a rank-1 broadcast of a_dst, and small per-chunk
  block-diagonal mask matmuls broadcasting a_src; exp(lrelu(x)) is computed
  as max(exp(x), exp(0.2 x)) (two ScalarE exps off PSUM + one cheap fp16 DVE
  max); per-graph matmuls of ex_g against node-major [h~ | 1] give aggregate
  + denominator in one pass; relu and the 1/den normalization fuse into one
  strided scalar_tensor_tensor per PSUM bank with a stride-0 broadcast AP.

  All matmul operands are fp16 (PSUM accumulation stays fp32): fp32 matmuls
  on TRN2 run 2-pass LOW_HIGH at 4 cycles/row, fp16 runs 1 cycle/row.
  Measured end-to-end error of the fp16 config vs the fp32 reference:
  ~3.5e-4 scale-relative.
"""

import sys

for _p in ("/opt/trn_rl_repo",):
    if _p not in sys.path:
        sys.path.append(_p)

import numpy as np

from contextlib import ExitStack

from concourse import bacc, bass, mybir, tile
from concourse.bass_types import AP
from concourse.bass_utils import run_bass_kernel_spmd

F32 = mybir.dt.float32
F16 = mybir.dt.float16
AF = mybir.ActivationFunctionType
ALU = mybir.AluOpType

NPG = 84            # nodes per graph
B = 256             # graphs
HID = 64
DEPTH = 3
NEG_SLOPE = 0.2
NC_CORES = 8
GPC = B // NC_CORES     # 32 graphs per core
NB = GPC * NPG          # 2688 nodes per core
CH = 448                # free-dim chunk (one PSUM bank)
NCH = NB // CH          # 6 chunks

# projection column layout: [a_dst | W(64) | ones | a_src | v(layer2)]
C_ADST, C_W0, C_ONE, C_ASRC, C_V = 0, 1, 65, 66, 67


def _chunk_graphs(c):
    """Graphs whose columns intersect chunk c."""
    g_lo = (CH * c) // NPG
    g_hi = (CH * (c + 1) - 1) // NPG
    return g_lo, min(g_hi, GPC - 1)


def _host_preprocess(inputs):
    x = np.ascontiguousarray(np.asarray(inputs['x'], np.float32))
    ei = np.asarray(inputs['edge_index'])
    ea = np.asarray(inputs['edge_attr'], np.float32)
    W0 = np.asarray(inputs['W0'], np.float32)
    Ws = np.asarray(inputs['Ws'], np.float32)
    asl = np.asarray(inputs['att_src_all'], np.float32)
    adl = np.asarray(inputs['att_dst_all'], np.float32)
    Wel = np.asarray(inputs['W_edge_all'], np.float32)
    ael = np.asarray(inputs['att_edge_all'], np.float32)
    bl = np.asarray(inputs['bias_all'], np.float32)
    linW = np.asarray(inputs['lin_W'], np.float32)
    linb = np.asarray(inputs['lin_b'], np.float32)

    src, dst = np.asarray(ei[0]), np.asarray(ei[1])
    g = src // NPG
    assert np.all(dst // NPG == g), "edges cross graph boundaries"
    sl, dl = src % NPG, dst % NPG

    dense = np.zeros((B, NPG, NPG, 2), np.float32)
    dense[g, sl, dl] = ea
    cnt = np.zeros((B, NPG), np.float32)
    np.add.at(cnt, (g, dl), 1.0)
    colsum = dense.sum(axis=1)
    loop_attr = colsum / np.maximum(cnt, 1.0)[..., None]
    di = np.arange(NPG)
    dense[:, di, di, :] = loop_attr

    Es = []
    for l in range(DEPTH):
        w2 = Wel[l] @ ael[l]
        Es.append(np.ascontiguousarray(dense @ w2, dtype=np.float16))

    W_all = [W0, Ws[0], Ws[1]]
    CW = []
    for l in range(DEPTH):
        K = W_all[l].shape[0]
        cols = [(W_all[l] @ adl[l])[:, None], W_all[l], np.zeros((K, 1), np.float32),
                (W_all[l] @ asl[l])[:, None]]
        if l == DEPTH - 1:
            cols.append(W_all[l] @ linW)
        A = np.concatenate(cols, axis=1)
        aug = np.zeros((1, A.shape[1]), np.float32)
        aug[0, C_ONE] = 1.0
        CW.append(np.ascontiguousarray(np.vstack([A, aug]), np.float16))

    tail_bias = float(NPG * float(bl[DEPTH - 1] @ linW[:, 0]) + float(linb[0]))

    # per-chunk block-diagonal masks: row k of chunk c covers graph g_lo(c)+k
    maskc = np.zeros((7, NB), np.float16)
    for c in range(NCH):
        g_lo, _ = _chunk_graphs(c)
        for j in range(CH):
            gg = (CH * c + j) // NPG
            maskc[gg - g_lo, CH * c + j] = 1.0
    ident = np.eye(NPG, dtype=np.float16)
    ones = np.ones((NPG, NPG), np.float16)
    x_aug = np.ones((2, B * NPG), np.float16)
    x_aug[0] = x[:, 0].astype(np.float16)

    return dict(x_aug=x_aug, Es=Es, CW=CW, bl=bl, tail_bias=tail_bias,
                maskc=maskc, ident=ident, ones=ones)


def _graph_banks(n_graphs, per_bank):
    out = []
    g0 = 0
    while g0 < n_graphs:
        out.append(list(range(g0, min(g0 + per_bank, n_graphs))))
        g0 += per_bank
    return out


def _bcast_inner(ap, n):
    """View `ap` with an extra innermost stride-0 axis of length n."""
    return AP(ap.tensor, ap.offset, list(ap.ap) + [[0, n]])


def _build_program(tail_bias, use_bias):
    """use_bias: (bool, bool) for layers 0 and 1 (per-node bias via ex@bb matmul)."""
    nc = bacc.Bacc("TRN2", target_bir_lowering=False, debug=False)

    xT_d = nc.dram_tensor("xT", [2, NB], F16, kind="ExternalInput").ap()
    E_d = [nc.dram_tensor(f"E{l}", [NPG, NB], F16, kind="ExternalInput").ap()
           for l in range(DEPTH)]
    ncw = [67, 67, 68]
    CW_d = [nc.dram_tensor(f"CW{l}", [(2 if l == 0 else HID + 1), ncw[l]],
                           F16, kind="ExternalInput").ap() for l in range(DEPTH)]
    maskc_d = nc.dram_tensor("maskc", [7, NB], F16, kind="ExternalInput").ap()
    ident_d = nc.dram_tensor("ident", [NPG, NPG], F16, kind="ExternalInput").ap()
    ones_d = nc.dram_tensor("ones", [NPG, NPG], F16, kind="ExternalInput").ap()
    bb_d = [nc.dram_tensor(f"bb{l}", [NPG, HID], F16, kind="ExternalInput").ap()
            if use_bias[l] else None for l in range(2)]
    # row bounce scratch (sbuf row -> dram -> repartitioned sbuf)
    asrc_tmp = [nc.dram_tensor(f"asrc_tmp{l}", [NB], F16).ap() for l in range(DEPTH)]
    v_tmp = nc.dram_tensor("v_tmp", [NB], F16).ap()
    q_tmp = nc.dram_tensor("q_tmp", [NPG * GPC], F32).ap()
    out_d = nc.dram_tensor("out", [GPC], F32, kind="ExternalOutput").ap()

    with tile.TileContext(nc) as tc, ExitStack() as ctx:
        cpool = ctx.enter_context(tc.tile_pool(name="const", bufs=1))
        hpool = ctx.enter_context(tc.tile_pool(name="h", bufs=2))
        ppool = ctx.enter_context(tc.tile_pool(name="proj", bufs=2))
        npool = ctx.enter_context(tc.tile_pool(name="hnode", bufs=2))
        expool = ctx.enter_context(tc.tile_pool(name="ex", bufs=2))
        echpool = ctx.enter_context(tc.tile_pool(name="ech", bufs=4))
        ltpool = ctx.enter_context(tc.tile_pool(name="lt", bufs=3))
        smpool = ctx.enter_context(tc.tile_pool(name="small", bufs=3))
        upool = ctx.enter_context(tc.tile_pool(name="u", bufs=2))

        psw = ctx.enter_context(tc.tile_pool(name="psw", bufs=2, space="PSUM"))
        psl = ctx.enter_context(tc.tile_pool(name="psl", bufs=2, space="PSUM"))
        psa = ctx.enter_context(tc.tile_pool(name="psa", bufs=2, space="PSUM"))
        pst = ctx.enter_context(tc.tile_pool(name="pst", bufs=1, space="PSUM"))
        pst2 = ctx.enter_context(tc.tile_pool(name="pst2", bufs=1, space="PSUM"))

        # constants
        maskc_sb = cpool.tile([7, NB], F16, tag="maskc")
        nc.sync.dma_start(maskc_sb[:], maskc_d[:])
        ident_sb = cpool.tile([NPG, NPG], F16, tag="ident")
        nc.sync.dma_start(ident_sb[:], ident_d[:])
        ones_sb = cpool.tile([NPG, NPG], F16, tag="ones")
        nc.sync.dma_start(ones_sb[:], ones_d[:])
        cw_sb = []
        for l in range(DEPTH):
            t = cpool.tile(list(CW_d[l].shape), F16, tag=f"cw{l}")
            nc.sync.dma_start(t[:], CW_d[l][:])
            cw_sb.append(t)
        bb_sb = []
        for l in range(2):
            if use_bias[l]:
                t = cpool.tile([NPG, HID], F16, tag=f"bb{l}")
                nc.sync.dma_start(t[:], bb_d[l][:])
                bb_sb.append(t)
            else:
                bb_sb.append(None)

        x_sb = hpool.tile([2, NB], F16, tag="x")
        nc.sync.dma_start(x_sb[:], xT_d[:])

        def projection_and_logits(l, hT_in, nrows):
            """Returns (pT, ex): projection tile [nrows, NB] and attention ex."""
            pT = ppool.tile([nrows, NB], F16, tag="pT")
            asrc_mats = []
            for c in range(NCH):
                cs = slice(c * CH, (c + 1) * CH)
                pw = psw.tile([nrows, CH], F32, tag="pw")
                nc.tensor.matmul(pw[:], cw_sb[l][:], hT_in[:, cs],
                                 start=True, stop=True)
                nc.scalar.copy(pT[:, cs], pw[:])
                # bounce this chunk's a_src row immediately
                nc.sync.dma_start(
                    asrc_tmp[l][cs].rearrange("(o n) -> o n", o=1),
                    pT[C_ASRC:C_ASRC + 1, cs])
            am_all = smpool.tile([7, NCH * NPG], F16, tag="asrc")
            for c in range(NCH):
                g_lo, g_hi = _chunk_graphs(c)
                ng = g_hi - g_lo + 1
                nc.sync.dma_start(
                    am_all[:ng, c * NPG:(c + 1) * NPG],
                    asrc_tmp[l][g_lo * NPG:(g_hi + 1) * NPG]
                    .rearrange("(g s) -> g s", g=ng))
                asrc_mats.append(am_all[:, c * NPG:(c + 1) * NPG])
            ex = expool.tile([NPG, NB], F16, tag="ex")
            for c in range(NCH):
                cs = slice(c * CH, (c + 1) * CH)
                g_lo, g_hi = _chunk_graphs(c)
                ng = g_hi - g_lo + 1
                ech = echpool.tile([NPG, CH], F16, tag="ech")
                nc.sync.dma_start(ech[:], E_d[l][:, cs])
                pl = psl.tile([NPG, CH], F32, tag="pl")
                nc.tensor.matmul(pl[:], ident_sb[:], ech[:],
                                 start=True, stop=False)
                nc.tensor.matmul(pl[:], ones_sb[0:1, :],
                                 pT[C_ADST:C_ADST + 1, cs],
                                 start=False, stop=False)
                nc.tensor.matmul(pl[:], asrc_mats[c][0:ng, :],
                                 maskc_sb[:ng, cs], start=False, stop=True)
                # exp(lrelu(x)) == max(exp(x), exp(0.2x))
                e1 = ltpool.tile([NPG, CH], F16, tag="e1")
                nc.scalar.activation(e1[:], pl[:], AF.Exp)
                e2 = ltpool.tile([NPG, CH], F16, tag="e2")
                nc.scalar.activation(e2[:], pl[:], AF.Exp, scale=NEG_SLOPE)
                nc.vector.tensor_tensor(ex[:, cs], e1[:], e2[:], ALU.max)
            return pT, ex

        hT_in = x_sb
        for l in range(2):
            pT, ex = projection_and_logits(l, hT_in, 67)
            # node-major [adst | h~ | 1] blocks via per-graph PE transposes
            hnode = npool.tile([NPG, GPC * 66], F16, tag="hnode")
            for gs in _graph_banks(GPC, 7):
                pt = pst.tile([NPG, 66 * len(gs)], F16, tag="pt")
                for j, g in enumerate(gs):
                    nc.tensor.transpose(pt[:, j * 66:(j + 1) * 66],
                                        pT[:66, g * NPG:(g + 1) * NPG],
                                        ident_sb[:66, :66])
                nc.scalar.copy(hnode[:, gs[0] * 66:(gs[-1] + 1) * 66], pt[:])
            # per-graph aggregation: [agg(64) | den] in one matmul
            dent = smpool.tile([NPG, GPC], F32, tag="dent")
            recip = smpool.tile([NPG, GPC], F32, tag="recip")
            UN = upool.tile([NPG, GPC * HID], F16, tag="UN")
            for gs in _graph_banks(GPC, 7):
                pa = psa.tile([NPG, 65 * len(gs)], F32, tag="pa")
                for j, g in enumerate(gs):
                    exg = ex[:, g * NPG:(g + 1) * NPG]
                    o0 = j * 65
                    nc.tensor.matmul(pa[:, o0:o0 + 65], exg,
                                     hnode[:, g * 66 + 1:g * 66 + 66],
                                     start=True, stop=not use_bias[l])
                    if use_bias[l]:
                        nc.tensor.matmul(pa[:, o0:o0 + 64], exg, bb_sb[l][:],
                                         start=False, stop=True)
                gsl = slice(gs[0], gs[-1] + 1)
                nc.vector.tensor_copy(dent[:, gsl], pa[:, 64::65])
                nc.vector.reciprocal(recip[:, gsl], dent[:, gsl])
                # UN = max(agg, 0) * (1/den), fused (stride-0 bcast)
                pa3 = pa[:].rearrange("p (g c) -> p g c", c=65)[:, :, 0:64]
                un3 = (UN[:, gs[0] * HID:(gs[-1] + 1) * HID]
                       .rearrange("p (g c) -> p g c", c=64))
                rb = _bcast_inner(recip[:, gsl], 64)
                nc.vector.scalar_tensor_tensor(un3, pa3, 0.0, rb,
                                               ALU.max, ALU.mult)
            # transpose pairs back to feature-major [65, NB] (row 64 = ones)
            hT_next = hpool.tile([HID + 1, NB], F16, tag="hT")
            nc.gpsimd.memset(hT_next[HID:HID + 1, :], 1.0)
            pair_banks = _graph_banks(GPC // 2, 6)   # 16 pairs, banks of 6
            for pb in pair_banks:
                ntr = len(pb)
                pt2 = pst2.tile([128, NPG * ntr], F16, tag="pt2")
                for t, pj in enumerate(pb):
                    nc.tensor.transpose(
                        pt2[:, t * NPG:(t + 1) * NPG],
                        UN[:, (2 * pj) * HID:(2 * pj + 2) * HID],
                        ident_sb[:])
                g0 = 2 * pb[0]
                dst = (hT_next[0:HID, :]
                       .rearrange("p (g s) -> p g s", s=NPG))
                src = pt2[:].rearrange("p (t s) -> p t s", s=NPG)
                nc.scalar.copy(dst[:, g0:g0 + 2 * ntr:2, :], src[0:HID])
                nc.vector.tensor_copy(dst[:, g0 + 1:g0 + 2 * ntr:2, :],
                                      src[HID:2 * HID])
            hT_in = hT_next

        # ---- layer 2 (readout folded in) ----
        pT, ex = projection_and_logits(2, hT_in, 68)
        nc.sync.dma_start(v_tmp.rearrange("(o n) -> o n", o=1),
                          pT[C_V:C_V + 1, :])
        v_mat = smpool.tile([NPG, GPC], F16, tag="vmat")
        nc.sync.dma_start(v_mat[:], v_tmp.rearrange("(g s) -> s g", g=GPC))
        # interleaved [v | 1] columns so each graph aggregates with one matmul
        vo = smpool.tile([NPG, 2 * GPC], F16, tag="vo")
        nc.vector.memset(vo[:], 1.0)
        nc.vector.tensor_copy(vo[:, 0::2], v_mat[:])

        pq = psa.tile([NPG, 2 * GPC], F32, tag="pa")
        for g in range(GPC):
            nc.tensor.matmul(pq[:, 2 * g:2 * g + 2],
                             ex[:, g * NPG:(g + 1) * NPG],
                             vo[:, 2 * g:2 * g + 2], start=True, stop=True)
        den2 = smpool.tile([NPG, GPC], F32, tag="dent")
        nc.vector.tensor_copy(den2[:], pq[:, 1::2])
        recip2 = smpool.tile([NPG, GPC], F32, tag="recip")
        nc.vector.reciprocal(recip2[:], den2[:])
        qsb = smpool.tile([NPG, GPC], F32, tag="qsb")
        nc.vector.tensor_mul(qsb[:], pq[:, 0::2], recip2[:])
        # exact fp32 pooling: bounce [d, g] -> [g, d], then free-axis reduce
        nc.sync.dma_start(q_tmp.rearrange("(s g) -> s g", g=GPC), qsb[:])
        qT = smpool.tile([GPC, NPG], F32, tag="qT")
        nc.sync.dma_start(qT[:], q_tmp.rearrange("(s g) -> g s", g=GPC))
        zcol = smpool.tile([GPC, 1], F32, tag="zcol")
        nc.vector.reduce_sum(zcol[:], qT[:], axis=mybir.AxisListType.X)
        zout = smpool.tile([GPC, 1], F32, tag="zout")
        nc.scalar.activation(zout[:], zcol[:], AF.Relu, bias=float(tail_bias))
        nc.sync.dma_start(out_d.rearrange("(g o) -> g o", o=1), zout[:])

    nc.compile()
    return nc


def _core_inputs(pre, c):
    m = {
        'xT': np.ascontiguousarray(pre['x_aug'][:, c * NB:(c + 1) * NB]),
        'maskc': pre['maskc'], 'ident': pre['ident'], 'ones': pre['ones'],
    }
    for l in range(DEPTH):
        m[f'E{l}'] = np.ascontiguousarray(
            np.transpose(pre['Es'][l][c * GPC:(c + 1) * GPC], (1, 0, 2))
            .reshape(NPG, NB))
        m[f'CW{l}'] = pre['CW'][l]
    for l in range(2):
        if np.any(pre['bl'][l] != 0):
            m[f'bb{l}'] = np.ascontiguousarray(
                np.tile(pre['bl'][l][None, :], (NPG, 1)).astype(np.float16))
    return m


def kernel(**inputs):
    pre = _host_preprocess(inputs)
    use_bias = tuple(bool(np.any(pre['bl'][l] != 0)) for l in range(2))
    nc = _build_program(pre['tail_bias'], use_bias)
    in_maps = [_core_inputs(pre, c) for c in range(NC_CORES)]
    res = run_bass_kernel_spmd(nc, in_maps, list(range(NC_CORES)))
    out = np.concatenate([np.asarray(res.results[c]['out'])
                          for c in range(NC_CORES)])
    return out.reshape(B, 1).astype(np.float32)
